# revision 1
# baseline (speedup 1.0000x reference)
"""Trainium2 Bass kernel for LocallyConnected2d (3x3, pad 1, unshared weights).

  out[b,o,h,w] = sum_{c,k} patches[b,c,k,h,w] * weight[0,o,c,h,w,k]
  x: [8,32,64,64] f32, weight: [1,64,32,64,64,9] f32 -> out: [8,64,64,64] f32

Sharding: H dim split across 8 cores (8 rows each). Each core's weight shard
(36 MiB) is streamed through the TensorE as the moving matmul operand; the
3x3 patches (built on-device from a host-padded x halo) are the stationary
operand. Per spatial location: out[b,:] (8x64) = P_l[288,8]^T @ W_l[288,64],
split into 3 chunks of K=96 accumulating in PSUM, with 4 locations packed
into one PSUM bank via TensorE column tiling (tile_position=(0,32t)).

Host-side relayout of the weight gives the device perfectly contiguous
DMA streams; the kernel is HBM-bandwidth bound (~36 MiB/core).
"""

import os
import sys

sys.path.insert(0, "/opt/trn_rl_repo")

from contextlib import ExitStack

import numpy as np

import concourse.bass as bass  # noqa: F401
import concourse.tile as tile
from concourse import bacc, mybir
from concourse.bass_utils import run_bass_kernel_spmd

F32 = mybir.dt.float32
BF16 = mybir.dt.bfloat16

B, C, O, H, W, K = 8, 32, 64, 64, 64, 9
NCORES = 8
HL = H // NCORES          # 8 spatial rows per core
LOCS = HL * W             # 512 locations per core
NJ = 3                    # contraction chunks (96 = 32c x 3k each)
GL = 32                   # locations per W-DMA group
NG = LOCS // GL           # 16 groups
NSUB = GL // 4            # 8 sub-groups of 4 locations (one PSUM bank each)

_CACHED = {}


def _build_nc(sim: bool = False, repeat: int = 1, variant: str = "full"):
    nc = bacc.Bacc("TRN2", target_bir_lowering=False, debug=False,
                   num_devices=NCORES)
    w_d = nc.dram_tensor("w", [NJ, 96, LOCS, O], BF16,
                         kind="ExternalInput").ap()
    x_d = nc.dram_tensor("x", [C, B, HL + 2, W + 2], F32,
                         kind="ExternalInput").ap()
    # out[t, b, g, sub, o] with location l = g*32 + sub*4 + t
    o_d = nc.dram_tensor("out", [4, B, NG, NSUB, O], F32,
                         kind="ExternalOutput").ap()

    with tile.TileContext(nc) as tc, ExitStack() as ctx:
        xpool = ctx.enter_context(tc.tile_pool(name="xpool", bufs=1))
        ppool = ctx.enter_context(tc.tile_pool(name="ppool", bufs=1))
        wpool = ctx.enter_context(tc.tile_pool(name="wpool", bufs=2))
        pspool = ctx.enter_context(tc.tile_pool(name="pspool", bufs=8,
                                                space="PSUM"))
        stpool = ctx.enter_context(tc.tile_pool(name="stpool", bufs=3))

        x_sb = xpool.tile([C, B, HL + 2, W + 2], F32, name="x_sb")
        nc.sync.dma_start(x_sb[:], x_d[:])

        # Patches: P[j][32*k_in + c, b, h, w] = x[c, b, h + k//3, w + k%3]
        # (k = 3j + k_in; offsets already include the +1 pad shift)
        P = []
        for j in range(NJ):
            Pj = ppool.tile([96, B, HL, W], BF16, name=f"P{j}")
            P.append(Pj)
            for k_in in range(3):
                k = 3 * j + k_in
                dh, dw = k // 3, k % 3
                nc.vector.tensor_copy(
                    Pj[32 * k_in: 32 * (k_in + 1)],
                    x_sb[:, :, dh: dh + HL, dw: dw + W],
                )

        def body():
            for g in range(NG):
                Wg = []
                for j in range(NJ):
                    Wj = wpool.tile([96, GL, O], BF16, name=f"Wt{j}",
                                    tag=f"Wt{j}")
                    Wg.append(Wj)
                    nc.sync.dma_start(Wj[:], w_d[j, :, g * GL:(g + 1) * GL, :])

                stage = stpool.tile([128, NSUB, O], F32, name="stage")
                for sub in range(NSUB):
                    ps = pspool.tile([128, O], F32, name="ps")
                    if sim:
                        nc.vector.memset(ps[:], 0)
                    njs = {"full": NJ, "mm1": 1, "dma": 0}[variant]
                    for j in range(njs):
                        for t in range(4):
                            li = sub * 4 + t
                            l = g * GL + li
                            nc.tensor.matmul(
                                ps[32 * t: 32 * t + B, :],
                                P[j][:, :, l // W, l % W],
                                Wg[j][:, li, :],
                                start=(j == 0),
                                stop=(j == njs - 1),
                                skip_group_check=True,
                                tile_position=(0, 32 * t),
                            )
                    nc.vector.tensor_copy(stage[:, sub, :], ps[:])

                for t in range(4):
                    nc.sync.dma_start(o_d[t, :, g], stage[32 * t: 32 * t + B])

        if repeat > 1:
            with tc.For_i(0, repeat, 1):
                body()
        else:
            body()

    nc.compile()
    return nc


def _shard(x: np.ndarray, weight: np.ndarray):
    # Device weight layout: w[j, p, l, o] with p = 32*k_in + c, k = 3j + k_in,
    # l = h_local*W + w.
    import ml_dtypes

    wt = weight[0].transpose(4, 1, 2, 3, 0)          # [K, C, H, W, O]
    wt = np.ascontiguousarray(wt).astype(ml_dtypes.bfloat16)
    wt = wt.reshape(NJ, 96, H, W, O)
    xp = np.pad(x, ((0, 0), (0, 0), (1, 1), (1, 1))).transpose(1, 0, 2, 3)
    xp = np.ascontiguousarray(xp)                    # [C, B, H+2, W+2]
    in_maps = []
    for i in range(NCORES):
        h0 = i * HL
        in_maps.append({
            "w": np.ascontiguousarray(wt[:, :, h0:h0 + HL]).reshape(
                NJ, 96, LOCS, O),
            "x": np.ascontiguousarray(xp[:, :, h0:h0 + HL + 2, :]),
        })
    return in_maps


def _gather(outs):
    full = np.empty((B, O, H, W), np.float32)
    for i, oc in enumerate(outs):                    # oc [4, B, NG, NSUB, O]
        tmp = oc.transpose(1, 2, 3, 0, 4).reshape(B, LOCS, O)   # [b, l, o]
        tmp = tmp.reshape(B, HL, W, O).transpose(0, 3, 1, 2)    # [b, o, h, w]
        full[:, :, i * HL:(i + 1) * HL, :] = tmp
    return full


def _get_nc():
    if "nc" not in _CACHED:
        _CACHED["nc"] = _build_nc()
    return _CACHED["nc"]


def kernel(**inputs) -> np.ndarray:
    x = np.ascontiguousarray(np.asarray(inputs["x"], dtype=np.float32))
    weight = np.asarray(inputs["weight"], dtype=np.float32)
    in_maps = _shard(x, weight)
    nc = _get_nc()
    res = run_bass_kernel_spmd(nc, in_maps, core_ids=list(range(NCORES)),
                               trace=bool(os.environ.get("BASS_TRACE_RUN")))
    if os.environ.get("BASS_TRACE_RUN"):
        _CACHED["last_results"] = res
    return _gather([r["out"] for r in res.results])



# revision 7
# speedup vs baseline: 2.5510x; 2.5510x over previous
"""Trainium2 Bass kernel for LocallyConnected2d (3x3, pad 1, unshared weights).

  out[b,o,h,w] = sum_{c,k} patches[b,c,k,h,w] * weight[0,o,c,h,w,k]
  x: [8,32,64,64] f32, weight: [1,64,32,64,64,9] f32 -> out: [8,64,64,64] f32

Sharding: H dim split across 8 cores (8 rows each, 512 locations). The
weight is the dominant HBM traffic (18.9 MiB/core in bf16); it streams
through TensorE as the moving operand. Contraction (c,k)=288 is chunked
128+128+32 so weight DMAs use all 128 partitions with 16 KiB contiguous
runs. Patches are prebuilt on host (bf16) and DMA'd once per iteration.
Per location l (col group t=l%4): out[8b,64o] += P[:,b]^T W[:,o], three
accumulating chunks into a full PSUM bank shared by 8 locations; one
[128,512] copy evicts the bank, one padded bf16 DMA stores it. DMA issue
alternates between the SP and ACT HWDGE rings.
"""

import os
import sys

sys.path.insert(0, "/opt/trn_rl_repo")

from contextlib import ExitStack

import numpy as np

import concourse.bass as bass  # noqa: F401
import concourse.tile as tile
from concourse import bacc, mybir
from concourse.bass_utils import run_bass_kernel_spmd

F32 = mybir.dt.float32
BF16 = mybir.dt.bfloat16

B, C, O, H, W, K = 8, 32, 64, 64, 64, 9
NCORES = 8
HL = H // NCORES          # 8 spatial rows per core
LOCS = HL * W             # 512 locations per core
GL = 128                  # locations per weight-DMA group
NG = LOCS // GL           # 4 groups
NBLK = 4                  # 8-location PSUM blocks per group
PFREE = B * HL * W        # patch free size per partition (4096)

_CACHED = {}


def _build_nc(sim: bool = False, repeat: int = 1, variant: str = "full"):
    nc = bacc.Bacc("TRN2", target_bir_lowering=False, debug=False,
                   num_devices=NCORES)
    wa_d = nc.dram_tensor("wa", [2, 128, LOCS, O], BF16,
                          kind="ExternalInput").ap()
    wb_d = nc.dram_tensor("wb", [32, LOCS, O], BF16,
                          kind="ExternalInput").ap()
    pa_d = nc.dram_tensor("pa", [2, 128, B, HL, W], BF16,
                          kind="ExternalInput").ap()
    pb_d = nc.dram_tensor("pb", [32, B, HL, W], BF16,
                          kind="ExternalInput").ap()
    # out[gb, 32t+b, s*64+o] for location l = gb*32 + s*4 + t
    o_d = nc.dram_tensor("out", [NG * NBLK, 128, 512], BF16,
                         kind="ExternalOutput").ap()

    with tile.TileContext(nc) as tc, ExitStack() as ctx:
        ppool = ctx.enter_context(tc.tile_pool(name="ppool", bufs=2))
        wpool = ctx.enter_context(tc.tile_pool(name="wpool", bufs=2))
        pspool = ctx.enter_context(tc.tile_pool(name="pspool", bufs=4,
                                                space="PSUM"))
        stpool = ctx.enter_context(tc.tile_pool(name="stpool", bufs=4))

        def body():
            P0 = ppool.tile([128, B, HL, W], BF16, name="P0", tag="P0")
            P1 = ppool.tile([128, B, HL, W], BF16, name="P1", tag="P1")
            P2 = ppool.tile([32, B, HL, W], BF16, name="P2", tag="P2")
            nc.scalar.dma_start(P0[:], pa_d[0])
            nc.scalar.dma_start(P1[:], pa_d[1])
            nc.scalar.dma_start(P2[:], pb_d[:])
            P = (P0, P1, P2)

            for g in range(NG):
                wa0 = wpool.tile([128, GL, O], BF16, name="wa0", tag="wa0")
                wa1 = wpool.tile([128, GL, O], BF16, name="wa1", tag="wa1")
                wb = wpool.tile([32, GL, O], BF16, name="wb", tag="wb")
                nc.sync.dma_start(wa0[:], wa_d[0, :, g * GL:(g + 1) * GL, :])
                nc.scalar.dma_start(wa1[:], wa_d[1, :, g * GL:(g + 1) * GL, :])
                nc.sync.dma_start(wb[:], wb_d[:, g * GL:(g + 1) * GL, :])
                Wt = (wa0, wa1, wb)

                for blk in range(NBLK):
                    ps = pspool.tile([128, 512], F32, name="ps")
                    if sim:
                        nc.vector.memset(ps[:], 0)
                    njs = {"full": 3, "mm1": 1, "dma": 0}[variant]
                    for s in range(8):
                        for j in range(njs):
                            for t in range(4):
                                li = (blk * 8 + s) * 4 + t
                                l = g * GL + li
                                nc.tensor.matmul(
                                    ps[32 * t: 32 * t + B,
                                       s * 64:(s + 1) * 64],
                                    P[j][:, :, l // W, l % W],
                                    Wt[j][:, li, :],
                                    start=(j == 0),
                                    stop=(j == njs - 1),
                                    skip_group_check=True,
                                    tile_position=(0, 32 * t),
                                )
                    stage = stpool.tile([128, 512], BF16, name="stage")
                    nc.vector.tensor_copy(stage[:], ps[:])
                    nc.sync.dma_start(o_d[g * NBLK + blk], stage[:])

        if repeat > 1:
            with tc.For_i(0, repeat, 1):
                body()
        else:
            body()

    nc.compile()
    return nc


def _shard(x: np.ndarray, weight: np.ndarray):
    """Host relayout: per-core weight chunks + prebuilt bf16 patches."""
    import ml_dtypes

    BF = ml_dtypes.bfloat16
    # w_flat[32k+c, h, w, o] = weight[0, o, c, h, w, k]
    # weight[0] is [O, C, H, W, K] -> transpose to [K, C, H, W, O]
    wt = np.ascontiguousarray(weight[0].transpose(4, 1, 2, 3, 0)).astype(BF)
    wt = wt.reshape(K * C, H, W, O)                  # row r = 32k + c
    xp = np.pad(x, ((0, 0), (0, 0), (1, 1), (1, 1)))  # [B, C, H+2, W+2]
    in_maps = []
    for i in range(NCORES):
        h0 = i * HL
        wl = wt[:, h0:h0 + HL].reshape(K * C, LOCS, O)
        patches = np.empty((K * C, B, HL, W), dtype=BF)
        for k in range(K):
            dh, dw = k // 3, k % 3
            win = xp[:, :, h0 + dh:h0 + dh + HL, dw:dw + W]   # [B,C,HL,W]
            patches[32 * k:32 * (k + 1)] = win.transpose(1, 0, 2, 3)
        in_maps.append({
            "wa": np.ascontiguousarray(wl[:256]).reshape(2, 128, LOCS, O),
            "wb": np.ascontiguousarray(wl[256:]),
            "pa": np.ascontiguousarray(patches[:256]).reshape(2, 128, B, HL, W),
            "pb": np.ascontiguousarray(patches[256:]),
        })
    return in_maps


def _gather(outs):
    full = np.empty((B, O, H, W), np.float32)
    for i, oc in enumerate(outs):            # oc [16, 128, 512] bf16
        oc = np.asarray(oc, np.float32).reshape(16, 4, 32, 8, 64)
        # [gb, t, b(<8 of 32), s, o]; l = gb*32 + s*4 + t
        oc = oc[:, :, :B]                    # [16, 4, 8, 8, 64]
        oc = oc.transpose(2, 4, 0, 3, 1)     # [b, o, gb, s, t]
        oc = oc.reshape(B, O, LOCS)          # l = (gb*8+s)*4+t
        full[:, :, i * HL:(i + 1) * HL, :] = oc.reshape(B, O, HL, W)
    return full


def _get_nc():
    if "nc" not in _CACHED:
        _CACHED["nc"] = _build_nc()
    return _CACHED["nc"]


def kernel(**inputs) -> np.ndarray:
    x = np.ascontiguousarray(np.asarray(inputs["x"], dtype=np.float32))
    weight = np.asarray(inputs["weight"], dtype=np.float32)
    in_maps = _shard(x, weight)
    nc = _get_nc()
    res = run_bass_kernel_spmd(nc, in_maps, core_ids=list(range(NCORES)),
                               trace=bool(os.environ.get("BASS_TRACE_RUN")))
    if os.environ.get("BASS_TRACE_RUN"):
        _CACHED["last_results"] = res
    return _gather([r["out"] for r in res.results])


# revision 42
# speedup vs baseline: 5.4408x; 2.1328x over previous
"""Trainium2 Bass kernel for LocallyConnected2d (3x3, pad 1, unshared weights).

  out[b,o,h,w] = sum_{c,k} patches[b,c,k,h,w] * weight[0,o,c,h,w,k]
  x: [8,32,64,64] f32, weight: [1,64,32,64,64,9] f32 -> out: [8,64,64,64] f32

Sharding: H dim split across 8 cores (8 rows each, 512 locations). The
weight is the dominant HBM traffic (18.9 MiB/core in bf16); it streams
through TensorE as the moving operand. Contraction (c,k)=288 is chunked
128+128+32 so weight DMAs use all 128 partitions with contiguous runs;
the K=32 tail is packed 4-locations-deep on 128 partitions and computed
at tile_position=(32t,32t) (row group == col group == l%4). Patches are
prebuilt on host (bf16) and DMA'd once per iteration. Weights stream in
512 KiB granules (32 locations) so TensorE never idles past the ~3.4 us
HAM window (fine granules beat 2 MiB tiles by ~9 us/iter on HW). Per
location l (col group t=l%4): out[8b,64o] += P[:,b]^T W[:,o], three
accumulating chunks (start on j0, stop on j2, s-major order -- a
single-start-per-bank scheme matching CoreSim's bank-wide has_written
clear computes GARBAGE on real HW) into a full PSUM bank shared by 8
locations; one [128,512] DVE copy evicts the bank, one padded bf16 DMA
stores it. Weight DMAs own the SP HWDGE
ring; patches/outputs ride the ACT ring so their semaphore waits never
stall weight prefetch. The timing loop uses For_i with PE branch-
prefetch hints and 4x body unrolling to amortize the ~2 us back-edge
barrier; deep pools (weights x6, all 8 PSUM banks) decouple the
DMA/PE/DVE pipelines.
"""

import os
import sys

sys.path.insert(0, "/opt/trn_rl_repo")

from contextlib import ExitStack

import numpy as np

import concourse.bass as bass  # noqa: F401
import concourse.tile as tile
from concourse import bacc, mybir
from concourse.bass_utils import run_bass_kernel_spmd

F32 = mybir.dt.float32
BF16 = mybir.dt.bfloat16

B, C, O, H, W, K = 8, 32, 64, 64, 64, 9
NCORES = 8
HL = H // NCORES          # 8 spatial rows per core
LOCS = HL * W             # 512 locations per core
GL = 32                   # locations per weight-DMA group (= one PSUM block)
NG = LOCS // GL           # 16 groups
PFREE = B * HL * W        # patch free size per partition (4096)

_CACHED = {}


def _build_nc(sim: bool = False, repeat: int = 1, variant: str = "full",
              staggered: bool = False, unroll: int = 1):
    nc = bacc.Bacc("TRN2", target_bir_lowering=False, debug=False,
                   num_devices=NCORES)
    wa_d = nc.dram_tensor("wa", [2, 128, LOCS, O], BF16,
                          kind="ExternalInput").ap()
    # wb packed on 128 partitions: wb[32*(l%4)+c, l//4, o]
    wb_d = nc.dram_tensor("wb", [128, LOCS // 4, O], BF16,
                          kind="ExternalInput").ap()
    pa_d = nc.dram_tensor("pa", [128, 2, B, HL, W], BF16,
                          kind="ExternalInput").ap()
    # tap-8 patches replicated across the 4 row groups (pb[32r+c] = p[c])
    pb_d = nc.dram_tensor("pb", [128, B, HL, W], BF16,
                          kind="ExternalInput").ap()
    # out[g, 32t+b, s*64+o] for location l = g*32 + s*4 + t
    o_d = nc.dram_tensor("out", [NG, 128, 512], BF16,
                         kind="ExternalOutput").ap()

    with tile.TileContext(nc) as tc, ExitStack() as ctx:
        ppool = ctx.enter_context(tc.tile_pool(name="ppool", bufs=2))
        wpool = ctx.enter_context(tc.tile_pool(name="wpool", bufs=6))
        pspool = ctx.enter_context(tc.tile_pool(name="pspool", bufs=8,
                                                space="PSUM"))
        stpool = ctx.enter_context(tc.tile_pool(name="stpool", bufs=4))

        def body(staged: bool = False):
            njs = {"full": 3, "mm2": 2, "mm1": 1, "dma": 0}[variant]
            P01 = ppool.tile([128, 2, B, HL, W], BF16, name="P01", tag="P01")
            P2 = ppool.tile([128, B, HL, W], BF16, name="P2", tag="P2")
            wb = ppool.tile([128, LOCS // 4, O], BF16, name="wb", tag="wb")
            nc.scalar.dma_start(wb[:], wb_d[:])
            nc.scalar.dma_start(P01[:], pa_d[:])
            nc.scalar.dma_start(P2[:], pb_d[:])

            for g in range(NG):
                if staged and g % 4 == 0 and g > 0:
                    tc.stage_boundary()
                wa0 = wpool.tile([128, GL, O], BF16, name="wa0", tag="wa0")
                wa1 = wpool.tile([128, GL, O], BF16, name="wa1", tag="wa1")
                nc.sync.dma_start(wa0[:], wa_d[0, :, g * GL:(g + 1) * GL, :])
                nc.sync.dma_start(wa1[:], wa_d[1, :, g * GL:(g + 1) * GL, :])

                ps = pspool.tile([128, 512], F32, name="ps")
                if sim or njs == 0:
                    nc.vector.memset(ps[:], 0)
                for s in range(8):
                    for j in range(njs):
                        for t in range(4):
                            li = s * 4 + t
                            l = g * GL + li
                            if j < 2:
                                lhsT = P01[:, j, :, l // W, l % W]
                                rhs = (wa0, wa1)[j][:, li, :]
                                tp = (0, 32 * t)
                            else:
                                # l % 4 == t, so row group == col group
                                lhsT = P2[32 * t:32 * (t + 1), :,
                                          l // W, l % W]
                                rhs = wb[32 * t:32 * (t + 1), l // 4, :]
                                tp = (32 * t, 32 * t)
                            nc.tensor.matmul(
                                ps[32 * t: 32 * t + B,
                                   s * 64:(s + 1) * 64],
                                lhsT,
                                rhs,
                                start=(j == 0),
                                stop=(j == njs - 1),
                                skip_group_check=True,
                                tile_position=tp,
                            )
                stage = stpool.tile([128, 512], BF16, name="stage")
                nc.vector.tensor_copy(stage[:], ps[:])
                nc.scalar.dma_start(o_d[g], stage[:])

        if repeat > 1:
            assert repeat % unroll == 0
            with tc.For_i(0, repeat // unroll, 1,
                          hint_engines=(mybir.EngineType.PE,),
                          staggered_reset=staggered):
                for _ in range(unroll):
                    body(staged=staggered)
        else:
            body()

    nc.compile()
    return nc


def _shard(x: np.ndarray, weight: np.ndarray):
    """Host relayout: per-core weight chunks + prebuilt bf16 patches."""
    import ml_dtypes

    BF = ml_dtypes.bfloat16
    # w_flat[32k+c, h, w, o] = weight[0, o, c, h, w, k]
    # weight[0] is [O, C, H, W, K] -> transpose to [K, C, H, W, O]
    wt = np.ascontiguousarray(weight[0].transpose(4, 1, 2, 3, 0)).astype(BF)
    wt = wt.reshape(K * C, H, W, O)                  # row r = 32k + c
    xp = np.pad(x, ((0, 0), (0, 0), (1, 1), (1, 1)))  # [B, C, H+2, W+2]
    in_maps = []
    for i in range(NCORES):
        h0 = i * HL
        wl = wt[:, h0:h0 + HL].reshape(K * C, LOCS, O)
        patches = np.empty((K * C, B, HL, W), dtype=BF)
        for k in range(K):
            dh, dw = k // 3, k % 3
            win = xp[:, :, h0 + dh:h0 + dh + HL, dw:dw + W]   # [B,C,HL,W]
            patches[32 * k:32 * (k + 1)] = win.transpose(1, 0, 2, 3)
        # wbr[32r+c, q, o] = wl[256+c, 4q+r, o]
        wbr = wl[256:].reshape(32, LOCS // 4, 4, O).transpose(2, 0, 1, 3)
        in_maps.append({
            "wa": np.ascontiguousarray(wl[:256]).reshape(2, 128, LOCS, O),
            "wb": np.ascontiguousarray(wbr).reshape(128, LOCS // 4, O),
            "pa": np.ascontiguousarray(
                patches[:256].reshape(2, 128, B, HL, W).transpose(1, 0, 2, 3, 4)),
            "pb": np.ascontiguousarray(
                np.concatenate([patches[256:]] * 4, axis=0)),
        })
    return in_maps


def _gather(outs):
    full = np.empty((B, O, H, W), np.float32)
    for i, oc in enumerate(outs):            # oc [16, 128, 512] bf16
        oc = np.asarray(oc, np.float32).reshape(16, 4, 32, 8, 64)
        # [gb, t, b(<8 of 32), s, o]; l = gb*32 + s*4 + t
        oc = oc[:, :, :B]                    # [16, 4, 8, 8, 64]
        oc = oc.transpose(2, 4, 0, 3, 1)     # [b, o, gb, s, t]
        oc = oc.reshape(B, O, LOCS)          # l = (gb*8+s)*4+t
        full[:, :, i * HL:(i + 1) * HL, :] = oc.reshape(B, O, HL, W)
    return full


def _get_nc():
    if "nc" not in _CACHED:
        _CACHED["nc"] = _build_nc()
    return _CACHED["nc"]


def kernel(**inputs) -> np.ndarray:
    x = np.ascontiguousarray(np.asarray(inputs["x"], dtype=np.float32))
    weight = np.asarray(inputs["weight"], dtype=np.float32)
    in_maps = _shard(x, weight)
    nc = _get_nc()
    res = run_bass_kernel_spmd(nc, in_maps, core_ids=list(range(NCORES)),
                               trace=bool(os.environ.get("BASS_TRACE_RUN")))
    if os.environ.get("BASS_TRACE_RUN"):
        _CACHED["last_results"] = res
    return _gather([r["out"] for r in res.results])


# revision 44
# speedup vs baseline: 5.6164x; 1.0323x over previous
"""Trainium2 Bass kernel for LocallyConnected2d (3x3, pad 1, unshared weights).

  out[b,o,h,w] = sum_{c,k} patches[b,c,k,h,w] * weight[0,o,c,h,w,k]
  x: [8,32,64,64] f32, weight: [1,64,32,64,64,9] f32 -> out: [8,64,64,64] f32

Sharding: H dim split across 8 cores (8 rows each, 512 locations). The
weight is the dominant HBM traffic (18.9 MiB/core in bf16); it streams
through TensorE as the moving operand. Contraction (c,k)=288 is chunked
128+128+32 so weight DMAs use all 128 partitions with contiguous runs;
the K=32 tail is packed 4-locations-deep on 128 partitions and computed
at tile_position=(32t,32t) (row group == col group == l%4). Patches are
prebuilt on host (bf16) and DMA'd once per iteration. Weights stream in
512 KiB granules (32 locations) so TensorE never idles past the ~3.4 us
HAM window (fine granules beat 2 MiB tiles by ~9 us/iter on HW). Per
location l (col group t=l%4): out[8b,64o] += P[:,b]^T W[:,o], three
accumulating chunks (start on j0, stop on j2, s-major order -- a
single-start-per-bank scheme matching CoreSim's bank-wide has_written
clear computes GARBAGE on real HW) into a full PSUM bank shared by 8
locations; one [128,512] DVE copy evicts the bank, one padded bf16 DMA
stores it. Weight DMAs own the SP HWDGE
ring; patches/outputs ride the ACT ring so their semaphore waits never
stall weight prefetch. The timing loop uses For_i with PE branch-
prefetch hints and 8x body unrolling to amortize the ~2 us back-edge
barrier; deep pools (weights x8, all 8 PSUM banks) decouple the
DMA/PE/DVE pipelines.
"""

import os
import sys

sys.path.insert(0, "/opt/trn_rl_repo")

from contextlib import ExitStack

import numpy as np

import concourse.bass as bass  # noqa: F401
import concourse.tile as tile
from concourse import bacc, mybir
from concourse.bass_utils import run_bass_kernel_spmd

F32 = mybir.dt.float32
BF16 = mybir.dt.bfloat16

B, C, O, H, W, K = 8, 32, 64, 64, 64, 9
NCORES = 8
HL = H // NCORES          # 8 spatial rows per core
LOCS = HL * W             # 512 locations per core
GL = 32                   # locations per weight-DMA group (= one PSUM block)
NG = LOCS // GL           # 16 groups
PFREE = B * HL * W        # patch free size per partition (4096)

_CACHED = {}


def _build_nc(sim: bool = False, repeat: int = 1, variant: str = "full",
              staggered: bool = False, unroll: int = 1):
    nc = bacc.Bacc("TRN2", target_bir_lowering=False, debug=False,
                   num_devices=NCORES)
    wa_d = nc.dram_tensor("wa", [2, 128, LOCS, O], BF16,
                          kind="ExternalInput").ap()
    # wb packed on 128 partitions: wb[32*(l%4)+c, l//4, o]
    wb_d = nc.dram_tensor("wb", [128, LOCS // 4, O], BF16,
                          kind="ExternalInput").ap()
    pa_d = nc.dram_tensor("pa", [128, 2, B, HL, W], BF16,
                          kind="ExternalInput").ap()
    # tap-8 patches replicated across the 4 row groups (pb[32r+c] = p[c])
    pb_d = nc.dram_tensor("pb", [128, B, HL, W], BF16,
                          kind="ExternalInput").ap()
    # out[g, 32t+b, s*64+o] for location l = g*32 + s*4 + t
    o_d = nc.dram_tensor("out", [NG, 128, 512], BF16,
                         kind="ExternalOutput").ap()

    with tile.TileContext(nc) as tc, ExitStack() as ctx:
        ppool = ctx.enter_context(tc.tile_pool(name="ppool", bufs=2))
        wpool = ctx.enter_context(tc.tile_pool(name="wpool", bufs=8))
        pspool = ctx.enter_context(tc.tile_pool(name="pspool", bufs=8,
                                                space="PSUM"))
        stpool = ctx.enter_context(tc.tile_pool(name="stpool", bufs=8))

        def body(staged: bool = False):
            njs = {"full": 3, "mm2": 2, "mm1": 1, "dma": 0}[variant]
            P01 = ppool.tile([128, 2, B, HL, W], BF16, name="P01", tag="P01")
            P2 = ppool.tile([128, B, HL, W], BF16, name="P2", tag="P2")
            wb = ppool.tile([128, LOCS // 4, O], BF16, name="wb", tag="wb")
            nc.scalar.dma_start(wb[:], wb_d[:])
            nc.scalar.dma_start(P01[:], pa_d[:])
            nc.scalar.dma_start(P2[:], pb_d[:])

            for g in range(NG):
                if staged and g % 4 == 0 and g > 0:
                    tc.stage_boundary()
                wa0 = wpool.tile([128, GL, O], BF16, name="wa0", tag="wa0")
                wa1 = wpool.tile([128, GL, O], BF16, name="wa1", tag="wa1")
                nc.sync.dma_start(wa0[:], wa_d[0, :, g * GL:(g + 1) * GL, :])
                nc.sync.dma_start(wa1[:], wa_d[1, :, g * GL:(g + 1) * GL, :])

                ps = pspool.tile([128, 512], F32, name="ps")
                if sim or njs == 0:
                    nc.vector.memset(ps[:], 0)
                for s in range(8):
                    for j in range(njs):
                        for t in range(4):
                            li = s * 4 + t
                            l = g * GL + li
                            if j < 2:
                                lhsT = P01[:, j, :, l // W, l % W]
                                rhs = (wa0, wa1)[j][:, li, :]
                                tp = (0, 32 * t)
                            else:
                                # l % 4 == t, so row group == col group
                                lhsT = P2[32 * t:32 * (t + 1), :,
                                          l // W, l % W]
                                rhs = wb[32 * t:32 * (t + 1), l // 4, :]
                                tp = (32 * t, 32 * t)
                            nc.tensor.matmul(
                                ps[32 * t: 32 * t + B,
                                   s * 64:(s + 1) * 64],
                                lhsT,
                                rhs,
                                start=(j == 0),
                                stop=(j == njs - 1),
                                skip_group_check=True,
                                tile_position=tp,
                            )
                stage = stpool.tile([128, 512], BF16, name="stage")
                nc.vector.tensor_copy(stage[:], ps[:])
                nc.scalar.dma_start(o_d[g], stage[:])

        if repeat > 1:
            assert repeat % unroll == 0
            with tc.For_i(0, repeat // unroll, 1,
                          hint_engines=(mybir.EngineType.PE,),
                          staggered_reset=staggered):
                for _ in range(unroll):
                    body(staged=staggered)
        else:
            body()

    nc.compile()
    return nc


def _shard(x: np.ndarray, weight: np.ndarray):
    """Host relayout: per-core weight chunks + prebuilt bf16 patches."""
    import ml_dtypes

    BF = ml_dtypes.bfloat16
    # w_flat[32k+c, h, w, o] = weight[0, o, c, h, w, k]
    # weight[0] is [O, C, H, W, K] -> transpose to [K, C, H, W, O]
    wt = np.ascontiguousarray(weight[0].transpose(4, 1, 2, 3, 0)).astype(BF)
    wt = wt.reshape(K * C, H, W, O)                  # row r = 32k + c
    xp = np.pad(x, ((0, 0), (0, 0), (1, 1), (1, 1)))  # [B, C, H+2, W+2]
    in_maps = []
    for i in range(NCORES):
        h0 = i * HL
        wl = wt[:, h0:h0 + HL].reshape(K * C, LOCS, O)
        patches = np.empty((K * C, B, HL, W), dtype=BF)
        for k in range(K):
            dh, dw = k // 3, k % 3
            win = xp[:, :, h0 + dh:h0 + dh + HL, dw:dw + W]   # [B,C,HL,W]
            patches[32 * k:32 * (k + 1)] = win.transpose(1, 0, 2, 3)
        # wbr[32r+c, q, o] = wl[256+c, 4q+r, o]
        wbr = wl[256:].reshape(32, LOCS // 4, 4, O).transpose(2, 0, 1, 3)
        in_maps.append({
            "wa": np.ascontiguousarray(wl[:256]).reshape(2, 128, LOCS, O),
            "wb": np.ascontiguousarray(wbr).reshape(128, LOCS // 4, O),
            "pa": np.ascontiguousarray(
                patches[:256].reshape(2, 128, B, HL, W).transpose(1, 0, 2, 3, 4)),
            "pb": np.ascontiguousarray(
                np.concatenate([patches[256:]] * 4, axis=0)),
        })
    return in_maps


def _gather(outs):
    full = np.empty((B, O, H, W), np.float32)
    for i, oc in enumerate(outs):            # oc [16, 128, 512] bf16
        oc = np.asarray(oc, np.float32).reshape(16, 4, 32, 8, 64)
        # [gb, t, b(<8 of 32), s, o]; l = gb*32 + s*4 + t
        oc = oc[:, :, :B]                    # [16, 4, 8, 8, 64]
        oc = oc.transpose(2, 4, 0, 3, 1)     # [b, o, gb, s, t]
        oc = oc.reshape(B, O, LOCS)          # l = (gb*8+s)*4+t
        full[:, :, i * HL:(i + 1) * HL, :] = oc.reshape(B, O, HL, W)
    return full


def _get_nc():
    if "nc" not in _CACHED:
        _CACHED["nc"] = _build_nc()
    return _CACHED["nc"]


def kernel(**inputs) -> np.ndarray:
    x = np.ascontiguousarray(np.asarray(inputs["x"], dtype=np.float32))
    weight = np.asarray(inputs["weight"], dtype=np.float32)
    in_maps = _shard(x, weight)
    nc = _get_nc()
    res = run_bass_kernel_spmd(nc, in_maps, core_ids=list(range(NCORES)),
                               trace=bool(os.environ.get("BASS_TRACE_RUN")))
    if os.environ.get("BASS_TRACE_RUN"):
        _CACHED["last_results"] = res
    return _gather([r["out"] for r in res.results])
